# revision 12
# baseline (speedup 1.0000x reference)
"""BitLinear (B=8) tensor-parallel Trainium2 kernel.

Reference computation (see problem):
    gamma = max(max|x|, 1e-5)                  # global over x
    xq    = clip(round(x * 256/gamma), -256, 255)
    beta  = max(mean|W|, 1e-5)                 # global over W
    wq    = clip(round(|W|/beta), -1, 1)       # in {0, 1}
    y     = (xq @ wq.T) * (beta*gamma/256)

Distribution: W rows (out_features) sharded across 8 cores (1376 per core),
x replicated; each core's shard is shipped column-major ([in, out]) so the
quantized weights land directly in the matmul's stationary layout.  The
gamma and beta partials are packed into ONE 2-element AllGather (their
data-ready times coincide, and back-to-back collectives serialize on the
CC stream at ~20us each).  The matmul runs in bf16: xq in [-256,255] and
wq in {0,1} are exact in bf16, and products/sums stay < 2^21 so fp32 PSUM
accumulation is exact.

Scheduling notes (HWDGE rings are strict FIFO per issuing engine, and each
engine's queue is strict FIFO, so issue order is everything):
  sync ring   : xg + W pass-1 interleaved 1:4 (pure nonblocking issues ->
                full-rate DMA; the abs/reduce consumers run on ACT/DVE),
                then the collective bounce hops, then x prefetch, then the
                per-tile xq transposes and the main x stream.
  gpsimd ring : W pass-2 re-read issues (consumers on ACT/DVE) + y out.
  DVE queue   : the pass-2 is_gt for k-tile k is emitted inside the main
                k-loop, so the x quantizes are not stuck behind 32 blocked
                is_gt ops in the FIFO.
The first two token tiles share one k-loop so each wqT k-tile arriving
from the pass-2 stream unlocks matmuls in six PSUM banks at once.

Quantization tricks (all f32-exact, matching jax semantics):
  round-half-even(v) == (v + 1.5*2^23) - 1.5*2^23   (fp32 RNE arithmetic)
  clip(round(v), ..., 255) == round(min(v, 255.49998...))
  wq == (|W| > 0.5*beta)   since round(u)>=1 iff u>0.5, and clip at 1
"""

import numpy as np

# ---- problem constants (hardcoded; kernel.py must be self-contained) ----
B_DIM, S_DIM, I_DIM, O_DIM = 4, 2048, 4096, 11008
N_CORES = 8
O_SHARD = O_DIM // N_CORES          # 1376 out-features per core
T_DIM = B_DIM * S_DIM               # 8192 tokens
TOK_SLICE = T_DIM // N_CORES        # 1024 tokens reduced per core for gamma

EPS = 1e-5
QVAL = 256.0
M_MAGIC = 12582912.0                # 1.5 * 2**23 : fp32 round-to-int magic
CLIP_HI = float(np.nextafter(np.float32(255.5), np.float32(0.0)))


def build_kernel(T=T_DIM, I=I_DIM, O_SH=O_SHARD, n_cores=N_CORES,
                 tok_slice=None, n_total=None):
    """Build + compile the SPMD Bass kernel. Returns the Bacc object.

    Inputs (per core): x [T, I] f32 (replicated), xg [tok_slice, I] f32
    (this core's token slice, for the gamma partial), wt [I, O_SH] f32
    (this core's weight shard, column-major).  Output: y [T, O_SH] f32.
    """
    import concourse.bacc as bacc
    import concourse.mybir as mybir
    import concourse.tile as tile
    from concourse import bass_isa
    from concourse.bass import ts

    if tok_slice is None:
        tok_slice = T // n_cores
    if n_total is None:
        n_total = float(O_DIM) * float(I_DIM)  # mean divisor (full W)

    f32 = mybir.dt.float32
    bf16 = mybir.dt.bfloat16
    Alu = mybir.AluOpType
    Act = mybir.ActivationFunctionType

    KT = I // 128              # k-tiles (contraction)
    ST = T // 128              # token tiles
    GT = tok_slice // 128      # gamma-slice tiles
    # matmul free-dim chunks over the output features (PSUM bank = 512 f32)
    ochunks = []
    off = 0
    while off < O_SH:
        w_ = min(512, O_SH - off)
        ochunks.append((off, w_))
        off += w_

    N_PRE = min(3, ST)         # x tiles prefetched during the stat phase
    N_RAMP = min(2, ST)        # token tiles k-interleaved during the ramp
    N_W2PRE = 7                # pass-2 W tiles buffered before beta lands

    nc = bacc.Bacc("TRN2", target_bir_lowering=False, debug=False,
                   num_devices=n_cores)

    x_d = nc.dram_tensor("x", [T, I], f32, kind="ExternalInput")
    xg_d = nc.dram_tensor("xg", [tok_slice, I], f32, kind="ExternalInput")
    wt_d = nc.dram_tensor("wt", [I, O_SH], f32, kind="ExternalInput")
    y_d = nc.dram_tensor("y", [T, O_SH], f32, kind="ExternalOutput")
    # collective bounce buffers: one fused 2-scalar AllGather
    shared = "Shared" if n_cores > 4 else "Local"
    ccd_in = nc.dram_tensor("ccd_in", [1], f32)
    ccd_out = nc.dram_tensor("ccd_out", [n_cores], f32, addr_space=shared)
    cc_in = nc.dram_tensor("cc_in", [2], f32)
    cc_out = nc.dram_tensor("cc_out", [2 * n_cores], f32, addr_space=shared)

    with tile.TileContext(nc) as tc:
        with (
            tc.tile_pool(name="big", bufs=3) as big_pool,     # [128, I] f32
            tc.tile_pool(name="wtp", bufs=N_W2PRE) as wt_pool,  # [128,O_SH]
            tc.tile_pool(name="bfq", bufs=1) as bfq_pool,     # [128, I] bf16
            tc.tile_pool(name="tp", bufs=2) as tp_pool,       # xqT tiles
            tc.tile_pool(name="wres", bufs=1) as wres_pool,   # resident wqT
            tc.tile_pool(name="stat", bufs=1) as stat_pool,   # stats/scalars
            tc.tile_pool(name="yout", bufs=1) as y_pool,      # [128,O_SH] f32
            tc.tile_pool(name="ps", bufs=8, space="PSUM") as ps_pool,
        ):
            wqT = wres_pool.tile([128, KT, O_SH], bf16)
            gmax = stat_pool.tile([128, 2 * GT], f32)
            wsum = stat_pool.tile([128, KT], f32)
            redw = stat_pool.tile([128, 1], f32)
            redg = stat_pool.tile([128, 1], f32)
            redwg = stat_pool.tile([128, 2], f32)
            sc1 = stat_pool.tile([1, 2 * n_cores], f32)
            scb = stat_pool.tile([128, 2 * n_cores], f32)
            scal = stat_pool.tile([128, 8], f32)
            n256 = stat_pool.tile([128, 1], f32)
            dscr = stat_pool.tile([1, 1], f32)

            # ---- CC warm-up: a dummy AllGather at t~0 so the real
            # collective starts with ~1us trigger latency instead of ~11us
            # (the CC stack takes ~85us to spin up either way, well before
            # the real collective's ~135us trigger).
            nc.vector.memset(dscr[:, :], 0.0)
            nc.sync.dma_start(ccd_in[:], dscr[0:1, 0:1])
            nc.gpsimd.collective_compute(
                "AllGather", Alu.bypass,
                replica_groups=[list(range(n_cores))],
                ins=[ccd_in.ap()], outs=[ccd_out.ap()])

            # ---- stat pass (sync ring): xg and W pass-1 interleaved so
            # both streams progress at full DMA rate; consumers elsewhere.
            w_per_g = max(1, (KT + 2 * GT - 1) // (2 * GT))
            wk = 0

            def emit_w1(k):
                wt_t = wt_pool.tile([128, O_SH], f32, tag="wtile",
                                    name="wt_t")
                nc.sync.dma_start(wt_t, wt_d[ts(k, 128), :])
                nc.vector.tensor_reduce(
                    wsum[:, k:k + 1], wt_t, axis=mybir.AxisListType.X,
                    op=Alu.add, apply_absolute_value=True)

            IH = I // 2
            for t in range(2 * GT):
                xg_t = big_pool.tile([128, IH], f32, tag="bigtile",
                                     name="xg_t")
                nc.sync.dma_start(
                    xg_t, xg_d[ts(t // 2, 128), (t % 2) * IH:(t % 2 + 1) * IH])
                nc.vector.tensor_reduce(
                    gmax[:, t:t + 1], xg_t, axis=mybir.AxisListType.X,
                    op=Alu.max, apply_absolute_value=True)
                for _ in range(w_per_g):
                    if wk < KT:
                        emit_w1(wk)
                        wk += 1
            while wk < KT:
                emit_w1(wk)
                wk += 1

            # pass-2 W prefetch (gpsimd ring; nonblocking, slots free as
            # pass-1 abs completes)
            w2_tiles = {}
            for k in range(min(N_W2PRE, KT)):
                wt2 = wt_pool.tile([128, O_SH], f32, tag="wtile",
                                   name="wt2")
                nc.scalar.dma_start(wt2, wt_d[ts(k, 128), :])
                w2_tiles[k] = wt2

            # ---- fused stat reduce + single AllGather (hops on sync) ----
            nc.vector.tensor_reduce(redw, wsum,
                                    axis=mybir.AxisListType.X, op=Alu.add)
            nc.vector.tensor_reduce(redg, gmax,
                                    axis=mybir.AxisListType.X, op=Alu.max)
            nc.gpsimd.partition_all_reduce(redwg[:, 0:1], redw, channels=128,
                                           reduce_op=bass_isa.ReduceOp.add)
            nc.gpsimd.partition_all_reduce(redwg[:, 1:2], redg, channels=128,
                                           reduce_op=bass_isa.ReduceOp.max)
            nc.sync.dma_start(cc_in[:], redwg[0:1, 0:2])
            nc.gpsimd.collective_compute(
                "AllGather", Alu.bypass,
                replica_groups=[list(range(n_cores))],
                ins=[cc_in.ap()], outs=[cc_out.ap()])
            nc.gpsimd.dma_start(dscr, ccd_out.ap().rearrange(
                "(a b) -> a b", a=1)[0:1, 0:1])

            # prefetch x tiles on the sync ring while the collective runs;
            # they hold big_pool slots until gamma lands
            pre_x = []
            for st in range(N_PRE):
                x_t = big_pool.tile([128, I], f32, tag="bigtile",
                                    name="x_pre")
                nc.sync.dma_start(x_t, x_d[ts(st, 128), :])
                pre_x.append(x_t)

            # cc_out layout [w0,g0,w1,g1,...]; contiguous readback, one
            # broadcast, then strided in-SBUF reduces (DVE handles the APs)
            nc.sync.dma_start(sc1, cc_out.ap().rearrange("(a b) -> a b",
                                                         a=1))
            nc.gpsimd.partition_broadcast(scb, sc1)
            scb_r = scb[:, :].rearrange("p (c b) -> p b c", b=2)
            # beta = sum * fl(1/n)  (DVE has no divide ALU op; <=1ulp vs /n)
            nc.vector.tensor_reduce(scal[:, 1:2].unsqueeze(1), scb_r[:, 0:1, :],
                                    axis=mybir.AxisListType.X, op=Alu.add)
            inv_n = float(np.float32(1.0) / np.float32(n_total))
            nc.vector.tensor_scalar_mul(scal[:, 2:3], scal[:, 1:2], inv_n)
            nc.vector.tensor_scalar_max(scal[:, 2:3], scal[:, 2:3], EPS)
            # half_beta = 0.5*beta (exact)
            nc.vector.tensor_scalar_mul(scal[:, 4:5], scal[:, 2:3], 0.5)
            # gamma
            nc.vector.tensor_reduce(scal[:, 0:1].unsqueeze(1), scb_r[:, 1:2, :],
                                    axis=mybir.AxisListType.X, op=Alu.max)
            nc.vector.tensor_scalar_max(scal[:, 0:1], scal[:, 0:1], EPS)
            # s_x = 256/gamma via hw reciprocal; x256 is an exact pow2 scale,
            # so if reciprocal is correctly rounded this equals fl(256/gamma)
            nc.vector.reciprocal(n256, scal[:, 0:1])
            nc.vector.tensor_scalar_mul(scal[:, 3:4], n256, QVAL)
            # c_out = (beta*gamma)/256 : fp32 mult then exact pow2 scale
            nc.vector.tensor_tensor(scal[:, 5:6], scal[:, 2:3], scal[:, 0:1],
                                    op=Alu.mult)
            nc.vector.tensor_scalar_mul(scal[:, 5:6], scal[:, 5:6],
                                        1.0 / 256.0)

            # ---- helpers ----
            def emit_w2(k):
                """Quantize one W k-tile into wqT (ACT abs + DVE is_gt)."""
                if k in w2_tiles:
                    wt2 = w2_tiles.pop(k)
                else:
                    wt2 = wt_pool.tile([128, O_SH], f32, tag="wtile",
                                       name="wt2")
                    nc.scalar.dma_start(wt2, wt_d[ts(k, 128), :])
                nc.scalar.activation(wt2, wt2, Act.Abs)
                nc.vector.tensor_scalar(wqT[:, k, :], wt2, scal[:, 4:5],
                                        None, op0=Alu.is_gt)

            def quantize_and_transpose(x_t):
                # v = min(x*s_x, 255.4999...)   (in place, f32)
                nc.vector.tensor_scalar(x_t, x_t, scal[:, 3:4], CLIP_HI,
                                        op0=Alu.mult, op1=Alu.min)
                # round-half-even via +M -M; output bf16 (exact small ints)
                xq_t = bfq_pool.tile([128, I], bf16, tag="bfqtile",
                                     name="xq_t")
                nc.vector.tensor_scalar(xq_t, x_t, M_MAGIC, M_MAGIC,
                                        op0=Alu.add, op1=Alu.subtract)
                xqT_t = tp_pool.tile([128, KT, 128], bf16, name="xqT_t")
                nc.sync.dma_start(xqT_t, xq_t, transpose=True)
                return xqT_t

            def mk_ps():
                return [ps_pool.tile([128, width], f32, tag="ps",
                                     name="ps_t")
                        for (_, width) in ochunks]

            def emit_epilogue(st, ps_list):
                y_t = y_pool.tile([128, O_SH], f32, name="y_t")
                for ci, (off, width) in enumerate(ochunks):
                    nc.scalar.activation(y_t[:, off:off + width],
                                         ps_list[ci][:, :], Act.Copy,
                                         scale=scal[:, 5:6])
                nc.gpsimd.dma_start(y_d[ts(st, 128), :], y_t)

            # ---- ramp: first N_RAMP token tiles share one k loop; the
            # pass-2 quantize for k-tile k is emitted right before the
            # matmuls that consume it ----
            ramp_xqT = [quantize_and_transpose(pre_x[st])
                        for st in range(N_RAMP)]
            ramp_ps = [mk_ps() for _ in range(N_RAMP)]
            for k in range(KT):
                emit_w2(k)
                for st in range(N_RAMP):
                    for ci, (off, width) in enumerate(ochunks):
                        nc.tensor.matmul(
                            ramp_ps[st][ci][:, :],
                            ramp_xqT[st][:, k, :],
                            wqT[:, k, off:off + width],
                            start=(k == 0), stop=(k == KT - 1))
            for st in range(N_RAMP):
                emit_epilogue(st, ramp_ps[st])

            # ---- steady state ----
            for st in range(N_RAMP, ST):
                if st < N_PRE:
                    x_t = pre_x[st]
                else:
                    x_t = big_pool.tile([128, I], f32, tag="bigtile",
                                        name="x_t")
                    nc.sync.dma_start(x_t, x_d[ts(st, 128), :])
                xqT_t = quantize_and_transpose(x_t)
                ps_list = mk_ps()
                for k in range(KT):
                    for ci, (off, width) in enumerate(ochunks):
                        nc.tensor.matmul(
                            ps_list[ci][:, :],
                            xqT_t[:, k, :],
                            wqT[:, k, off:off + width],
                            start=(k == 0), stop=(k == KT - 1))
                emit_epilogue(st, ps_list)

    nc.compile()
    return nc


_CACHED_NC = None


def _get_nc():
    global _CACHED_NC
    if _CACHED_NC is None:
        _CACHED_NC = build_kernel()
    return _CACHED_NC


def shard_inputs(x, weight):
    """Host-side sharding/marshalling: full inputs -> per-core input maps."""
    x2 = np.ascontiguousarray(x.reshape(T_DIM, I_DIM).astype(np.float32,
                                                             copy=False))
    weight = weight.astype(np.float32, copy=False)
    in_maps = []
    for c in range(N_CORES):
        in_maps.append({
            "x": x2,
            "xg": np.ascontiguousarray(
                x2[c * TOK_SLICE:(c + 1) * TOK_SLICE]),
            "wt": np.ascontiguousarray(
                weight[c * O_SHARD:(c + 1) * O_SHARD].T),
        })
    return in_maps


def unshard_output(results):
    """Per-core y [T, O_SHARD] -> full y [B, S, O]."""
    parts = [results[c]["y"] for c in range(N_CORES)]
    return np.concatenate(parts, axis=1).reshape(B_DIM, S_DIM, O_DIM)


def run_on_cores(x, weight, trace=False):
    from concourse.bass_utils import run_bass_kernel_spmd
    nc = _get_nc()
    in_maps = shard_inputs(x, weight)
    res = run_bass_kernel_spmd(nc, in_maps, core_ids=list(range(N_CORES)),
                               trace=trace)
    return res


def kernel(x, weight):
    res = run_on_cores(x, weight, trace=False)
    return unshard_output(res.results)
